# revision 1
# baseline (speedup 1.0000x reference)
"""Trainium2 Bass kernel v2 for the encoder block (fp8 DoubleRow attention).

Strategy: data-parallel over batch (1 element/core, no collectives).
qkvr/v/proj matmuls run fp8e4m3 with DoubleRow (K=256/pass); energy,
att@v and the FFN stay bf16 (fp8 there would blow the 2e-2 budget).
Softmax normalization uses DVE reciprocal (no Ln/Exp table thrash);
exp is batched per psum tile; psum evacuations are spread across
ACT/DVE/Pool.

Scale bookkeeping (host-side): x8 = fp8(x) unscaled; wq8/wk8/wr8/wv8/
wproj8 = fp8(32*w).  q,k evac multiply by 1/32 (true values, bf16);
r evac keeps the 32x scale; v evac multiplies by 1/(32*sqrt(E)) so
v_aug holds v/sqrt(E); the v_aug "ones" column is 1/2 so recip gives
2/sum; og8 = P*(32 r)*(2/sum) = 64*og_true (fp8); proj psum is
64*32*(og@wproj) -> dequant 1/2048 fused into the residual add.
"""

import sys

if "/opt/trn_rl_repo" not in sys.path:
    sys.path.insert(0, "/opt/trn_rl_repo")

from contextlib import ExitStack

import numpy as np
import ml_dtypes

import concourse.bass as bass
import concourse.mybir as mybir
import concourse.tile as tile
from concourse import bacc
from concourse.bass_utils import run_bass_kernel_spmd
from concourse.masks import make_identity

F32 = mybir.dt.float32
BF16 = mybir.dt.bfloat16
FP8 = mybir.dt.float8e4
AF = mybir.ActivationFunctionType
ALU = mybir.AluOpType
DR = mybir.MatmulPerfMode.DoubleRow

N_CORES = 8
B, N, E = 8, 1024, 768
H, D = 8, 96
C = 4 * E
NQT = N // 128
NEC = E // 128
NCT = C // 128
LN_EPS = 1e-5
ESPL = [(0, 512), (512, 256)]

WS = 32.0            # weight pre-scale before fp8 quantization
INV_WS = 1.0 / WS
ONES_VAL = 0.5       # v_aug ones column value
PROJ_DEQ = 1.0 / (64.0 * 32.0)   # og8=64*og_true, wproj8=32*wproj


def _bcast_dma(nc, out_ap, row_ap):
    src = bass.AP(
        tensor=row_ap.tensor,
        offset=row_ap.offset,
        ap=[[0, out_ap.shape[0]], list(row_ap.ap[-1])],
    )
    nc.gpsimd.dma_start(out=out_ap, in_=src)


def _ln_stats_norm(nc, pool, t1, out, eps_t, g_bc, b_bc, identity_ln, tag):
    """LN over free dim 768 of t1 -> out (t1 may equal out's source)."""
    scr = pool.tile([128, 32], F32, tag=f"lns_{tag}", name=f"lns_{tag}")
    st = scr[:, 0:18].rearrange("p (a b) -> p a b", a=3)
    mv = scr[:, 24:26]
    rstd = scr[:, 26:27]
    t2 = out if identity_ln else pool.tile([128, E], F32, tag=f"lnt2_{tag}", name=f"lnt2_{tag}")
    for sg in range(3):
        nc.vector.bn_stats(st[:, sg, :], t1[:, sg * 256 : (sg + 1) * 256])
    nc.vector.bn_aggr(mv, st)
    nc.scalar.activation(out=rstd, in_=mv[:, 1:2], func=AF.Sqrt, bias=eps_t[:], scale=1.0)
    nc.vector.reciprocal(rstd, rstd)
    nc.vector.tensor_scalar(
        out=t2[:] if t2 is not out else t2, in0=t1, scalar1=mv[:, 0:1], scalar2=rstd,
        op0=ALU.subtract, op1=ALU.mult,
    )
    if not identity_ln:
        nc.vector.tensor_tensor(out=t2[:], in0=t2[:], in1=g_bc, op=ALU.mult)
        nc.vector.tensor_tensor(out=out, in0=t2[:], in1=b_bc, op=ALU.add)


def _build(identity_ln=False, zero_bias=False):
    nc = bacc.Bacc(num_devices=N_CORES)

    x_d = nc.declare_dram_parameter("x", [N, E], F32, isOutput=False)
    # fp8 DoubleRow stationaries: [128, h, t(q/k/r), kchunk, 2, 96]
    wqkr_d = nc.declare_dram_parameter("wqkr", [128, H, 3, 3, 2, D], FP8, isOutput=False)
    # fp8 moving weights: [128, kchunk, 2, 768]
    wv_d = nc.declare_dram_parameter("wv", [128, 3, 2, E], FP8, isOutput=False)
    # proj moving: [96, headpair, 2, 768]
    wproj_d = nc.declare_dram_parameter("wproj", [D, 4, 2, E], FP8, isOutput=False)
    bqkr_d = nc.declare_dram_parameter("bqkr", [D, 3, H], F32, isOutput=False)
    bv_d = nc.declare_dram_parameter("bv", [1, E], F32, isOutput=False)
    bproj_d = nc.declare_dram_parameter("bproj", [1, E], F32, isOutput=False)
    ln1g_d = nc.declare_dram_parameter("ln1g", [1, E], F32, isOutput=False)
    ln1b_d = nc.declare_dram_parameter("ln1b", [1, E], F32, isOutput=False)
    wff1_d = nc.declare_dram_parameter("wff1", [128, NCT, 3, 2, 128], FP8, isOutput=False)
    bff1_d = nc.declare_dram_parameter("bff1", [128, NCT], F32, isOutput=False)
    wff2_d = nc.declare_dram_parameter("wff2", [C, E], BF16, isOutput=False)
    bff2_d = nc.declare_dram_parameter("bff2", [1, E], F32, isOutput=False)
    ln2g_d = nc.declare_dram_parameter("ln2g", [1, E], F32, isOutput=False)
    ln2b_d = nc.declare_dram_parameter("ln2b", [1, E], F32, isOutput=False)
    y_d = nc.declare_dram_parameter("y", [N, E], F32, isOutput=True)

    with tile.TileContext(nc) as tc, ExitStack() as ctx:
        persist = ctx.enter_context(tc.tile_pool(name="persist", bufs=1))
        xt_pool = ctx.enter_context(tc.tile_pool(name="xt", bufs=1))
        x1_pool = ctx.enter_context(tc.tile_pool(name="x1", bufs=1))
        x1t_pool = ctx.enter_context(tc.tile_pool(name="x1t", bufs=1))
        vaug_pool = ctx.enter_context(tc.tile_pool(name="vaug", bufs=1))
        og_pool = ctx.enter_context(tc.tile_pool(name="og", bufs=1))

        identb = persist.tile([128, 128], BF16)
        make_identity(nc, identb[:])
        identf = persist.tile([128, 128], F32)
        make_identity(nc, identf[:])
        warm_t = persist.tile([128, 128], BF16)
        nc.vector.memset(warm_t[:], 0.0)
        with tc.tile_pool(name="warm_ps", bufs=2, space="PSUM") as warm_ps:
            for _ in range(12):
                wp_ = warm_ps.tile([128, 128], F32, tag="wp_", name="wp_")
                nc.tensor.matmul(wp_[:], warm_t[:], warm_t[:], start=True, stop=True)
                nc.tensor.matmul(wp_[:], warm_t[:], warm_t[:], start=True, stop=True)
        eps_t = persist.tile([128, 1], F32)
        nc.vector.memset(eps_t[:], LN_EPS)
        bqkr_t = persist.tile([D, 3, H], F32)
        nc.sync.dma_start(out=bqkr_t[:], in_=bqkr_d[:])
        # small fp8 weights up-front so phase B/D never wait on the x loads
        wv8 = persist.tile([128, 3, 2, E], FP8, name="wv8")
        nc.sync.dma_start(out=wv8[:], in_=wv_d[:])
        wp8 = persist.tile([D, 4, 2, E], FP8, name="wp8")
        nc.sync.dma_start(out=wp8[:], in_=wproj_d[:])

        # ---- phase A: load x, cast fp8, transpose -> x8T [128, ec, 1024] ----
        x8T = persist.tile([128, NEC, N], FP8, name="x8T")
        with (
            tc.tile_pool(name="xload", bufs=4) as xl_pool,
            tc.tile_pool(name="tp_ps", bufs=4, space="PSUM") as tp_ps,
        ):
            for qt in range(NQT):
                xl = xl_pool.tile([128, E], F32, tag="xl", name="xl")
                nc.sync.dma_start(out=xl[:], in_=x_d[qt * 128 : (qt + 1) * 128, :])
                xb = xl_pool.tile([128, E], BF16, tag="xb", name="xb")
                nc.vector.tensor_copy(xb[:], xl[:])
                pt = tp_ps.tile([128, NEC, 128], BF16, tag="pt", name="pt")
                for ec in range(NEC):
                    nc.tensor.transpose(pt[:, ec, :], xb[:, ec * 128 : (ec + 1) * 128], identb[:])
                if qt % 2 == 0:
                    nc.vector.tensor_copy(x8T[:, :, qt * 128 : (qt + 1) * 128], pt[:])
                else:
                    nc.scalar.activation(
                        out=x8T[:, :, qt * 128 : (qt + 1) * 128], in_=pt[:], func=AF.Copy,
                    )

        # ---- phase B: v = x @ wv (fp8 DR) -> v_aug [128, h, kt, 97] bf16 ----
        v_aug = vaug_pool.tile([128, H, NQT, D + 1], BF16, name="v_aug")
        nc.vector.memset(v_aug[:, :, :, D : D + 1], ONES_VAL)
        with (
            tc.tile_pool(name="bcv", bufs=1) as bcv_pool,
            tc.tile_pool(name="v_ps", bufs=3, space="PSUM") as v_ps,
        ):
            bv_bc = None
            if not zero_bias:
                bv_bc = bcv_pool.tile([128, E], F32, tag="bv", name="bv_bc")
                _bcast_dma(nc, bv_bc[:], bv_d[0:1, :])
            VSC = 1.0 / (WS * float(np.sqrt(E)))
            for qt in range(NQT):
                vps = v_ps.tile([128, E], F32, tag="vp", name="vp")
                # nb outer: interleaving accumulation groups within a psum
                # bank is illegal (start=True zeroes the whole 2KB region)
                for nb in range(3):
                    for kc in range(3):
                        nc.tensor.matmul(
                            vps[:, nb * 256 : (nb + 1) * 256],
                            x8T[:, 2 * kc : 2 * kc + 2, qt * 128 : (qt + 1) * 128],
                            wv8[:, kc, :, nb * 256 : (nb + 1) * 256],
                            start=(kc == 0), stop=(kc == 2), perf_mode=DR,
                        )
                for h in range(H):
                    dst = v_aug[:, h, qt, 0:D]
                    src = vps[:, h * D : (h + 1) * D]
                    if zero_bias:
                        nc.vector.tensor_scalar(
                            out=dst, in0=src, scalar1=VSC, scalar2=None, op0=ALU.mult,
                        )
                    else:
                        nc.vector.scalar_tensor_tensor(
                            out=dst, in0=src, scalar=VSC,
                            in1=bv_bc[:, h * D : (h + 1) * D],
                            op0=ALU.mult, op1=ALU.add,
                        )

        # ---- phase C: attention per head ----
        og8 = og_pool.tile([D, H, N], FP8, name="og8")
        with (
            tc.tile_pool(name="wqkr", bufs=2) as wqkr_pool,
            tc.tile_pool(name="qkr", bufs=2) as qkr_pool,
            tc.tile_pool(name="expET", bufs=2) as exp_pool,
            tc.tile_pool(name="att_tmp", bufs=2) as tmp_pool,
            tc.tile_pool(name="qkr_ps", bufs=2, space="PSUM") as qkr_ps,
            tc.tile_pool(name="eng_ps", bufs=2, space="PSUM") as eng_ps,
            tc.tile_pool(name="att_ps", bufs=1, space="PSUM") as att_ps,
        ):
            for h in range(H):
                w_sb = wqkr_pool.tile([128, 3, 3, 2, D], FP8, tag="w_qkr", name="w_qkr")
                nc.sync.dma_start(out=w_sb[:], in_=wqkr_d[:, h])
                qkrT = {}
                for si, name in enumerate(("q", "k", "r")):
                    dst = qkr_pool.tile([D, N], BF16, tag=f"{name}T", name=f"{name}T")
                    qkrT[name] = dst
                    for half in range(2):
                        ps = qkr_ps.tile([D, 512], F32, tag="qkrp", name="qkrp")
                        for sb in range(2):
                            for kc in range(3):
                                nc.tensor.matmul(
                                    ps[:, sb * 256 : (sb + 1) * 256],
                                    w_sb[:, si, kc],
                                    x8T[:, 2 * kc : 2 * kc + 2,
                                        half * 512 + sb * 256 : half * 512 + (sb + 1) * 256],
                                    start=(kc == 0), stop=(kc == 2), perf_mode=DR,
                                )
                        out_sl = dst[:, half * 512 : (half + 1) * 512]
                        if name == "r":
                            # keep 32x scale in r (folded into og8)
                            if zero_bias:
                                nc.vector.tensor_copy(out_sl, ps[:])
                            else:
                                nc.vector.tensor_scalar(
                                    out=out_sl, in0=ps[:],
                                    scalar1=bqkr_t[:, si, h : h + 1], scalar2=None,
                                    op0=ALU.add,
                                )
                        elif name == "k":
                            # ACT: Copy(x*scale + bias)
                            nc.scalar.activation(
                                out=out_sl, in_=ps[:], func=AF.Copy,
                                bias=0.0 if zero_bias else bqkr_t[:, si, h : h + 1],
                                scale=INV_WS,
                            )
                        else:
                            if zero_bias:
                                nc.vector.tensor_scalar(
                                    out=out_sl, in0=ps[:],
                                    scalar1=INV_WS, scalar2=None, op0=ALU.mult,
                                )
                            else:
                                nc.vector.tensor_scalar(
                                    out=out_sl, in0=ps[:],
                                    scalar1=INV_WS, scalar2=bqkr_t[:, si, h : h + 1],
                                    op0=ALU.mult, op1=ALU.add,
                                )
                # energy + exp: psum [128, 1024] per kt; one exp per kt -> bf16
                expET = exp_pool.tile([128, NQT, N], BF16, tag="expET", name="expET")
                for kt in range(NQT):
                    ep = eng_ps.tile([128, N], F32, tag="ep", name="ep")
                    for qh in range(2):
                        nc.tensor.matmul(
                            ep[:, qh * 512 : (qh + 1) * 512],
                            qkrT["k"][:, kt * 128 : (kt + 1) * 128],
                            qkrT["q"][:, qh * 512 : (qh + 1) * 512],
                            start=True, stop=True, skip_group_check=True,
                        )
                    nc.scalar.activation(
                        out=expET[:, kt, :], in_=ep[:], func=AF.Exp,
                    )
                # att@v with ones column; normalize+gate -> og8 (64x scale)
                op_ = att_ps.tile([D + 1, N], F32, tag="op", name="op")
                # kt outer: v_aug stationary reused for both qh halves
                for kt in range(NQT):
                    for qh in range(2):
                        nc.tensor.matmul(
                            op_[:, qh * 512 : (qh + 1) * 512],
                            v_aug[:, h, kt, :],
                            expET[:, kt, qh * 512 : (qh + 1) * 512],
                            start=(kt == 0), stop=(kt == NQT - 1),
                            skip_group_check=True,
                        )
                # early psum reads (sums + gated) free att psum quickly;
                # the divide/broadcast tail runs on gpsimd, off DVE/ACT.
                sums, gateds = [], []
                for qh in range(2):
                    sl = slice(qh * 512, (qh + 1) * 512)
                    su = tmp_pool.tile([1, 512], F32, tag=f"sums{qh}", name=f"sums{qh}")
                    nc.vector.tensor_copy(su[:], op_[D : D + 1, sl])
                    gated = tmp_pool.tile([D, 512], F32, tag=f"gated{qh}", name=f"gated{qh}")
                    nc.vector.tensor_tensor(
                        out=gated[:], in0=op_[0:D, sl],
                        in1=qkrT["r"][:, sl], op=ALU.mult,
                    )
                    sums.append(su)
                    gateds.append(gated)
                for qh in range(2):
                    sl = slice(qh * 512, (qh + 1) * 512)
                    rcp = tmp_pool.tile([1, 512], F32, tag=f"rcp{qh}", name=f"rcp{qh}")
                    nc.vector.reciprocal_approx_fast(rcp[:], sums[qh][:])
                    bcr = tmp_pool.tile([D, 512], F32, tag=f"bcr{qh}", name=f"bcr{qh}")
                    nc.gpsimd.partition_broadcast(bcr[:], rcp[:])
                    nc.vector.tensor_tensor(
                        out=og8[:, h, sl], in0=gateds[qh][:], in1=bcr[:], op=ALU.mult,
                    )

        # ---- phase D: proj (fp8 DR, head-pair contraction) + LN1 + x1T ----
        x1_all = x1_pool.tile([128, NQT, E], F32, name="x1_all")
        x1T8 = x1t_pool.tile([128, NEC, N], FP8, name="x1T8")
        with (
            tc.tile_pool(name="xr", bufs=3) as xr_pool,
            tc.tile_pool(name="bcmid", bufs=1) as bcm_pool,
            tc.tile_pool(name="ln_tmp", bufs=2) as ln_pool,
            tc.tile_pool(name="y1_ps", bufs=2, space="PSUM") as y1_ps,
            tc.tile_pool(name="tp1_ps", bufs=2, space="PSUM") as tp1_ps,
        ):
            bcm = None
            if not (identity_ln and zero_bias):
                bcm = bcm_pool.tile([128, 3, E], F32, name="bcm")
                for i, d in enumerate((bproj_d, ln1g_d, ln1b_d)):
                    _bcast_dma(nc, bcm[:, i, :], d[0:1, :])
            for qt in range(NQT):
                yp = y1_ps.tile([128, E], F32, tag="yp", name="yp")
                for nb in range(3):
                    for hp in range(4):
                        nc.tensor.matmul(
                            yp[:, nb * 256 : (nb + 1) * 256],
                            og8[:, 2 * hp : 2 * hp + 2, qt * 128 : (qt + 1) * 128],
                            wp8[:, hp, :, nb * 256 : (nb + 1) * 256],
                            start=(hp == 0), stop=(hp == 3), perf_mode=DR,
                        )
                xr = xr_pool.tile([128, E], F32, tag="xr", name="xr")
                nc.sync.dma_start(out=xr[:], in_=x_d[qt * 128 : (qt + 1) * 128, :])
                if not zero_bias:
                    nc.vector.tensor_tensor(out=xr[:], in0=xr[:], in1=bcm[:, 0, :], op=ALU.add)
                t1 = ln_pool.tile([128, E], F32, tag="t1", name="t1")
                nc.vector.scalar_tensor_tensor(
                    out=t1[:], in0=yp[:], scalar=PROJ_DEQ, in1=xr[:],
                    op0=ALU.mult, op1=ALU.add,
                )
                _ln_stats_norm(nc, ln_pool, t1[:], x1_all[:, qt, :], eps_t,
                               bcm[:, 1, :] if bcm is not None else None,
                               bcm[:, 2, :] if bcm is not None else None,
                               identity_ln, "ln1")
                pt1 = tp1_ps.tile([128, NEC, 128], F32, tag="pt1", name="pt1")
                for ec in range(NEC):
                    nc.tensor.transpose(pt1[:, ec, :], x1_all[:, qt, ec * 128 : (ec + 1) * 128], identf[:])
                if qt % 2 == 0:
                    nc.vector.tensor_copy(x1T8[:, :, qt * 128 : (qt + 1) * 128], pt1[:])
                else:
                    nc.scalar.activation(
                        out=x1T8[:, :, qt * 128 : (qt + 1) * 128], in_=pt1[:], func=AF.Copy,
                    )

        # ---- phase E: ffn + residual + LN2 -> y ----
        with (
            tc.tile_pool(name="bcend", bufs=1) as bce_pool,
            tc.tile_pool(name="gstore", bufs=1) as gs_pool,
            tc.tile_pool(name="wff1", bufs=6) as wf1_pool,
            tc.tile_pool(name="wff2", bufs=4) as wf2_pool,
            tc.tile_pool(name="ln2_tmp", bufs=2) as ln2_pool,
            tc.tile_pool(name="out", bufs=2) as out_pool,
        ):
            bce = None
            if not (identity_ln and zero_bias):
                bce = bce_pool.tile([128, 3, E], F32, name="bce")
                for i, d in enumerate((bff2_d, ln2g_d, ln2b_d)):
                    _bcast_dma(nc, bce[:, i, :], d[0:1, :])
            bff1_t = bce_pool.tile([128, NCT], F32, name="bff1_t")
            nc.sync.dma_start(out=bff1_t[:], in_=bff1_d[:])
            for half in range(2):
                gT = gs_pool.tile([128, NCT, 512], BF16, tag="gT", name="gT")
                with tc.tile_pool(name="h1_ps", bufs=4, space="PSUM") as h1_ps:
                    for ct in range(NCT):
                        w1 = wf1_pool.tile([128, 3, 2, 128], FP8, tag="w1", name="w1")
                        nc.sync.dma_start(out=w1[:], in_=wff1_d[:, ct])
                        hp_ = h1_ps.tile([128, 512], F32, tag="h1", name="h1")
                        for nb in range(2):
                            for kc in range(3):
                                nc.tensor.matmul(
                                    hp_[:, nb * 256 : (nb + 1) * 256],
                                    w1[:, kc],
                                    x1T8[:, 2 * kc : 2 * kc + 2,
                                         half * 512 + nb * 256 : half * 512 + (nb + 1) * 256],
                                    start=(kc == 0), stop=(kc == 2), perf_mode=DR,
                                )
                        nc.scalar.activation(
                            out=gT[:, ct, :], in_=hp_[:], func=AF.Gelu,
                            bias=bff1_t[:, ct : ct + 1], scale=INV_WS,
                        )
                with tc.tile_pool(name="y2_ps", bufs=1, space="PSUM") as y2_ps:
                    y2p = [y2_ps.tile([128, E], F32, tag=f"y2_{iq}", name=f"y2_{iq}_{half}")
                           for iq in range(4)]
                    for ct in range(NCT):
                        w2 = wf2_pool.tile([128, E], BF16, tag="w2", name="w2")
                        nc.sync.dma_start(out=w2[:], in_=wff2_d[ct * 128 : (ct + 1) * 128, :])
                        for iq in range(4):
                            for o, w in ESPL:
                                nc.tensor.matmul(
                                    y2p[iq][:, o : o + w],
                                    gT[:, ct, iq * 128 : (iq + 1) * 128],
                                    w2[:, o : o + w],
                                    start=(ct == 0), stop=(ct == NCT - 1),
                                )
                    for iq in range(4):
                        qt = half * 4 + iq
                        x1q = x1_all[:, qt, :]
                        if not zero_bias:
                            nc.vector.tensor_tensor(out=x1q, in0=x1q, in1=bce[:, 0, :], op=ALU.add)
                        nc.vector.tensor_tensor(out=x1q, in0=y2p[iq][:], in1=x1q, op=ALU.add)
                    for iq in range(4):
                        qt = half * 4 + iq
                        yout = out_pool.tile([128, E], F32, tag="yout", name="yout")
                        _ln_stats_norm(nc, ln2_pool, x1_all[:, qt, :], yout[:], eps_t,
                                       bce[:, 1, :] if bce is not None else None,
                                       bce[:, 2, :] if bce is not None else None,
                                       identity_ln, "ln2")
                        nc.sync.dma_start(out=y_d[qt * 128 : (qt + 1) * 128, :], in_=yout[:])

    nc.compile()
    return nc


_NC_CACHE = {}


def _get_nc(identity_ln=False, zero_bias=False):
    key = (identity_ln, zero_bias)
    if key not in _NC_CACHE:
        _NC_CACHE[key] = _build(identity_ln, zero_bias)
    return _NC_CACHE[key]


def _q8(a):
    return np.asarray(a, dtype=ml_dtypes.float8_e4m3fn)


def _prep_weights(w_qkvr, b_qkvr, w_proj, b_proj, ln1_g, ln1_b,
                  w_ff1, b_ff1, w_ff2, b_ff2, ln2_g, ln2_b):
    w4 = np.asarray(w_qkvr, np.float32).reshape(E, H, D, 4)
    b4 = np.asarray(b_qkvr, np.float32).reshape(H, D, 4)
    s = np.float32(1.0 / np.sqrt(E))

    # wqkr: [p, h, t, kc, i, d] = 32*w_t[256kc+128i+p, h, :]
    wqkr = np.zeros((128, H, 3, 3, 2, D), np.float32)
    for ti, t in enumerate((0, 1, 3)):
        wt = w4[..., t] * WS  # [E, H, D]
        for kc in range(3):
            for i in range(2):
                wqkr[:, :, ti, kc, i, :] = wt[256 * kc + 128 * i : 256 * kc + 128 * (i + 1)]
    # wv: [p, kc, i, n] = 32*wv[256kc+128i+p, n]
    wvf = w4[..., 2].reshape(E, E) * WS
    wv = np.zeros((128, 3, 2, E), np.float32)
    for kc in range(3):
        for i in range(2):
            wv[:, kc, i, :] = wvf[256 * kc + 128 * i : 256 * kc + 128 * (i + 1)]
    # wproj: [p(d), hp, i, n] = 32*wproj[(2hp+i)*96+p, n]
    wpf = np.asarray(w_proj, np.float32) * WS
    wp = np.zeros((D, 4, 2, E), np.float32)
    for hp in range(4):
        for i in range(2):
            wp[:, hp, i, :] = wpf[(2 * hp + i) * D : (2 * hp + i + 1) * D]
    # biases: q,k true; r pre-scaled by 32 (r keeps 32x)
    bqkr = np.stack([b4[..., 0], b4[..., 1], b4[..., 3] * WS], 0).transpose(2, 0, 1)
    bv = np.ascontiguousarray((b4[..., 2] * s).reshape(1, E))
    # wff1: [p, ct, kc, i, c] = 32*w_ff1[256kc+128i+p, 128ct+c]
    w1f = np.asarray(w_ff1, np.float32) * WS
    wff1 = np.zeros((128, NCT, 3, 2, 128), np.float32)
    w1r = w1f.reshape(3, 2, 128, NCT, 128)  # [kc, i, p, ct, c]
    wff1[:] = w1r.transpose(2, 3, 0, 1, 4)
    bff1 = np.ascontiguousarray(np.asarray(b_ff1, np.float32).reshape(NCT, 128).T)
    return {
        "wqkr": _q8(wqkr), "wv": _q8(wv), "wproj": _q8(wp),
        "bqkr": np.ascontiguousarray(bqkr), "bv": bv,
        "bproj": np.asarray(b_proj, np.float32).reshape(1, E).copy(),
        "ln1g": np.asarray(ln1_g, np.float32).reshape(1, E).copy(),
        "ln1b": np.asarray(ln1_b, np.float32).reshape(1, E).copy(),
        "wff1": _q8(wff1), "bff1": bff1,
        "wff2": np.ascontiguousarray(np.asarray(w_ff2, np.float32)).astype(ml_dtypes.bfloat16),
        "bff2": np.asarray(b_ff2, np.float32).reshape(1, E).copy(),
        "ln2g": np.asarray(ln2_g, np.float32).reshape(1, E).copy(),
        "ln2b": np.asarray(ln2_b, np.float32).reshape(1, E).copy(),
    }


def _in_maps(inputs):
    x = np.asarray(inputs["x"], np.float32)
    shared = _prep_weights(
        inputs["w_qkvr"], inputs["b_qkvr"], inputs["w_proj"], inputs["b_proj"],
        inputs["ln1_g"], inputs["ln1_b"], inputs["w_ff1"], inputs["b_ff1"],
        inputs["w_ff2"], inputs["b_ff2"], inputs["ln2_g"], inputs["ln2_b"],
    )
    return [{**shared, "x": np.ascontiguousarray(x[i])} for i in range(N_CORES)]


def _flags(inputs):
    z = lambda k: not np.any(np.asarray(inputs[k]))
    one = lambda k: bool(np.all(np.asarray(inputs[k]) == 1.0))
    identity_ln = (one("ln1_g") and z("ln1_b") and one("ln2_g") and z("ln2_b"))
    zero_bias = (z("b_qkvr") and z("b_proj") and z("b_ff2"))
    return identity_ln, zero_bias


def kernel(**inputs) -> np.ndarray:
    identity_ln, zero_bias = _flags(inputs)
    nc = _get_nc(identity_ln, zero_bias)
    res = run_bass_kernel_spmd(nc, _in_maps(inputs), core_ids=list(range(N_CORES)))
    return np.stack([res.results[i]["y"] for i in range(N_CORES)], axis=0)



# revision 10
# speedup vs baseline: 1.1978x; 1.1978x over previous
"""Trainium2 Bass kernel v3 for the encoder block.

Data-parallel over batch (1 element/core, no collectives).

Key points vs v2:
- x is pre-transposed + fp8-quantized host-side (x8T DMA'd directly);
  a bf16 copy of x is shipped for the residual. No on-chip phase A.
- All fp8 matmuls use N=512 moving tiles (half the instructions).
- att@v runs fp8 DoubleRow over key-tile pairs: exp is emitted as
  fp8 with a global -12 shift (max energy ~17.3 so exp' <= ~200 < 448,
  no overflow-NaN), v_aug is fp8 at 2048/(32*sqrt(E)) scale with a
  32.0 ones column; sums get a 1e-6 eps guard so a fully-underflowed
  row yields 0, never inf.
- Attention is software-pipelined: tensor order per head is
  qkr(h), attv(h-1), energy(h), hiding the ACT exp latency.
- FFN: wff2 kept resident in SBUF; ff2 runs iq-outer (one token tile's
  full contraction at a time) so LN2+store pipeline behind the matmuls
  instead of serializing at the end.

Scale bookkeeping (host-side): x8T = fp8(x)^T unscaled; wq8/wk8/wr8/
wv8/wproj8 = fp8(32*w); wff1 = fp8(32*w).  q,k evac multiply by 1/32
(true values, bf16); r evac keeps the 32x scale; v evac scales by
2048/(32*sqrt(E)) -> v_aug8 = fp8(2048*v/sqrt(E)), ones col = 32.
att psum: d cols = 2048/sqrt(E)*e^-12*(P@v); ones = 32*e^-12*sum(P).
og8 = (psum_d * r_sb) / (32*psum_one) = 64*(r*og)_true (fp8);
proj psum = 64*32*(og@wproj) -> dequant 1/2048 fused into residual.
"""

import sys

if "/opt/trn_rl_repo" not in sys.path:
    sys.path.insert(0, "/opt/trn_rl_repo")

from contextlib import ExitStack

import numpy as np
import ml_dtypes

import concourse.bass as bass
import concourse.mybir as mybir
import concourse.tile as tile
from concourse import bacc
from concourse.bass_utils import run_bass_kernel_spmd
from concourse.masks import make_identity

F32 = mybir.dt.float32
BF16 = mybir.dt.bfloat16
FP8 = mybir.dt.float8e4
AF = mybir.ActivationFunctionType
ALU = mybir.AluOpType
DR = mybir.MatmulPerfMode.DoubleRow

N_CORES = 8
B, N, E = 8, 1024, 768
H, D = 8, 96
C = 4 * E
NQT = N // 128
NEC = E // 128
NCT = C // 128
LN_EPS = 1e-5

WS = 32.0            # weight pre-scale before fp8 quantization
INV_WS = 1.0 / WS
C_SHIFT = 12.0       # exp(e - C_SHIFT): emax ~17.3 -> exp' <= ~210 < 448
ONES_VAL = 32.0      # v_aug ones column value
VA_SCALE = 2048.0    # v_aug8 = fp8(VA_SCALE * v_true / sqrt(E))
SUM_EPS = 1e-6
PROJ_DEQ = 1.0 / (64.0 * 32.0)   # og8=64*(r*og)_true, wproj8=32*wproj


def _bcast_dma(nc, out_ap, row_ap):
    src = bass.AP(
        tensor=row_ap.tensor,
        offset=row_ap.offset,
        ap=[[0, out_ap.shape[0]], list(row_ap.ap[-1])],
    )
    nc.gpsimd.dma_start(out=out_ap, in_=src)


def _ln_stats_norm(nc, pool, t1, out, eps_t, g_bc, b_bc, identity_ln, tag):
    """LN over free dim 768 of t1 -> out."""
    scr = pool.tile([128, 32], F32, tag=f"lns_{tag}", name=f"lns_{tag}")
    st = scr[:, 0:18].rearrange("p (a b) -> p a b", a=3)
    mv = scr[:, 24:26]
    rstd = scr[:, 26:27]
    t2 = out if identity_ln else pool.tile([128, E], F32, tag=f"lnt2_{tag}", name=f"lnt2_{tag}")
    for sg in range(3):
        nc.vector.bn_stats(st[:, sg, :], t1[:, sg * 256 : (sg + 1) * 256])
    nc.vector.bn_aggr(mv, st)
    nc.scalar.activation(out=rstd, in_=mv[:, 1:2], func=AF.Sqrt, bias=eps_t[:], scale=1.0)
    nc.vector.reciprocal(rstd, rstd)
    nc.vector.tensor_scalar(
        out=t2[:] if t2 is not out else t2, in0=t1, scalar1=mv[:, 0:1], scalar2=rstd,
        op0=ALU.subtract, op1=ALU.mult,
    )
    if not identity_ln:
        nc.vector.tensor_tensor(out=t2[:], in0=t2[:], in1=g_bc, op=ALU.mult)
        nc.vector.tensor_tensor(out=out, in0=t2[:], in1=b_bc, op=ALU.add)


def _build(identity_ln=False, zero_bias=False):
    nc = bacc.Bacc(num_devices=N_CORES)

    # host-prepped inputs
    x8t_d = nc.declare_dram_parameter("x8t", [128, NEC, N], FP8, isOutput=False)
    xb16_d = nc.declare_dram_parameter("xb16", [N, E], BF16, isOutput=False)
    # fp8 DoubleRow stationaries: [128, h, t(q/k/r), kchunk, 2, 96]
    wqkr_d = nc.declare_dram_parameter("wqkr", [128, H, 3, 3, 2, D], FP8, isOutput=False)
    wv_d = nc.declare_dram_parameter("wv", [128, 3, 2, E], FP8, isOutput=False)
    wproj_d = nc.declare_dram_parameter("wproj", [D, 4, 2, E], FP8, isOutput=False)
    bqkr_d = nc.declare_dram_parameter("bqkr", [D, 3, H], F32, isOutput=False)
    bv_d = nc.declare_dram_parameter("bv", [1, E], F32, isOutput=False)
    bproj_d = nc.declare_dram_parameter("bproj", [1, E], F32, isOutput=False)
    ln1g_d = nc.declare_dram_parameter("ln1g", [1, E], F32, isOutput=False)
    ln1b_d = nc.declare_dram_parameter("ln1b", [1, E], F32, isOutput=False)
    wff1_d = nc.declare_dram_parameter("wff1", [128, NCT, 3, 2, 128], FP8, isOutput=False)
    bff1_d = nc.declare_dram_parameter("bff1", [128, NCT], F32, isOutput=False)
    wff2_d = nc.declare_dram_parameter("wff2", [C, E], BF16, isOutput=False)
    bff2_d = nc.declare_dram_parameter("bff2", [1, E], F32, isOutput=False)
    ln2g_d = nc.declare_dram_parameter("ln2g", [1, E], F32, isOutput=False)
    ln2b_d = nc.declare_dram_parameter("ln2b", [1, E], F32, isOutput=False)
    y_d = nc.declare_dram_parameter("y", [N, E], F32, isOutput=True)

    with tile.TileContext(nc) as tc, ExitStack() as ctx:
        persist = ctx.enter_context(tc.tile_pool(name="persist", bufs=1))
        vaug_pool = ctx.enter_context(tc.tile_pool(name="vaug", bufs=1))
        og_pool = ctx.enter_context(tc.tile_pool(name="og", bufs=1))
        x1_pool = ctx.enter_context(tc.tile_pool(name="x1", bufs=1))
        x1t_pool = ctx.enter_context(tc.tile_pool(name="x1t", bufs=1))

        identb = persist.tile([128, 128], BF16)
        make_identity(nc, identb[:])
        eps_t = persist.tile([128, 1], F32)
        nc.vector.memset(eps_t[:], LN_EPS)
        shift_t = persist.tile([128, 1], F32)
        nc.vector.memset(shift_t[:], -C_SHIFT)
        bqkr_t = persist.tile([D, 3, H], F32)
        nc.sync.dma_start(out=bqkr_t[:], in_=bqkr_d[:])

        # big persistent DMA loads up front
        x8T = persist.tile([128, NEC, N], FP8, name="x8T")
        for ec in range(NEC):
            nc.sync.dma_start(out=x8T[:, ec, :], in_=x8t_d[:, ec, :])
        xb16 = persist.tile([128, NQT, E], BF16, name="xb16")
        for qt in range(NQT):
            nc.sync.dma_start(out=xb16[:, qt, :], in_=xb16_d[qt * 128 : (qt + 1) * 128, :])
        wv8 = persist.tile([128, 3, 2, E], FP8, name="wv8")
        nc.sync.dma_start(out=wv8[:], in_=wv_d[:])
        wp8 = persist.tile([D, 4, 2, E], FP8, name="wp8")
        nc.sync.dma_start(out=wp8[:], in_=wproj_d[:])
        wff1sb = persist.tile([128, NCT, 3, 2, 128], FP8, name="wff1sb")
        for cq in range(4):
            nc.sync.dma_start(out=wff1sb[:, cq * 6 : (cq + 1) * 6], in_=wff1_d[:, cq * 6 : (cq + 1) * 6])
        bff1_t = persist.tile([128, NCT], F32, name="bff1_t")
        nc.sync.dma_start(out=bff1_t[:], in_=bff1_d[:])

        # PE warm-up while DMAs land
        warm_t = persist.tile([128, 128], BF16)
        nc.vector.memset(warm_t[:], 0.0)
        with tc.tile_pool(name="warm_ps", bufs=2, space="PSUM") as warm_ps:
            for _ in range(8):
                wp_ = warm_ps.tile([128, 128], F32, tag="wp_", name="wp_")
                nc.tensor.matmul(wp_[:], warm_t[:], warm_t[:], start=True, stop=True)
                nc.tensor.matmul(wp_[:], warm_t[:], warm_t[:], start=True, stop=True)

        # ---- phase B: v = x @ wv (fp8 DR) -> v_aug8 [128, h, kt, 97] fp8 ----
        v_aug = vaug_pool.tile([128, H, NQT, 128], FP8, name="v_aug")
        nc.vector.memset(v_aug[:], 0.0)
        nc.vector.memset(v_aug[:, :, :, D : D + 1], ONES_VAL)
        VSC = VA_SCALE / (WS * float(np.sqrt(E)))
        with (
            tc.tile_pool(name="bcv", bufs=1) as bcv_pool,
            tc.tile_pool(name="v_ps", bufs=2, space="PSUM") as v_ps,
        ):
            bv_bc = None
            if not zero_bias:
                bv_bc = bcv_pool.tile([128, E], F32, tag="bv", name="bv_bc")
                _bcast_dma(nc, bv_bc[:], bv_d[0:1, :])
            for qt in range(NQT):
                vps = v_ps.tile([128, 1024], F32, tag="vp", name="vp")
                for o, w in ((0, 512), (512, 256)):
                    for kc in range(3):
                        nc.tensor.matmul(
                            vps[:, o : o + w],
                            x8T[:, 2 * kc : 2 * kc + 2, qt * 128 : (qt + 1) * 128],
                            wv8[:, kc, :, o : o + w],
                            start=(kc == 0), stop=(kc == 2), perf_mode=DR,
                        )
                dst = v_aug[:, :, qt, 0:D]
                src = vps[:, 0:E].rearrange("p (h d) -> p h d", h=H)
                if zero_bias:
                    nc.vector.tensor_scalar(
                        out=dst, in0=src, scalar1=VSC, scalar2=None, op0=ALU.mult,
                    )
                else:
                    nc.vector.scalar_tensor_tensor(
                        out=dst, in0=src, scalar=VSC,
                        in1=bv_bc[:, 0:E].rearrange("p (h d) -> p h d", h=H),
                        op0=ALU.mult, op1=ALU.add,
                    )

        # ---- phase C: attention, software-pipelined over heads ----
        og8 = og_pool.tile([D, H, N], FP8, name="og8")
        with (
            tc.tile_pool(name="wqkr", bufs=2) as wqkr_pool,
            tc.tile_pool(name="qkr", bufs=2) as qkr_pool,
            tc.tile_pool(name="expE", bufs=2) as exp_pool,
            tc.tile_pool(name="att_tmp", bufs=2) as tmp_pool,
            tc.tile_pool(name="qkr_ps", bufs=2, space="PSUM") as qkr_ps,
            tc.tile_pool(name="eng_ps", bufs=2, space="PSUM") as eng_ps,
            tc.tile_pool(name="att_ps", bufs=1, space="PSUM") as att_ps,
        ):
            state = {}  # per-head tiles carried across the pipeline

            def qkr_stage(h):
                w_sb = wqkr_pool.tile([128, 3, 3, 2, D], FP8, tag="w_qkr", name="w_qkr")
                nc.sync.dma_start(out=w_sb[:], in_=wqkr_d[:, h])
                qkrT = {}
                for si, name in enumerate(("q", "k", "r")):
                    dst = qkr_pool.tile([D, N], BF16, tag=f"{name}T", name=f"{name}T")
                    qkrT[name] = dst
                    for half in range(2):
                        ps = qkr_ps.tile([D, 512], F32, tag="qkrp", name="qkrp")
                        for kc in range(3):
                            nc.tensor.matmul(
                                ps[:],
                                w_sb[:, si, kc],
                                x8T[:, 2 * kc : 2 * kc + 2,
                                    half * 512 : (half + 1) * 512],
                                start=(kc == 0), stop=(kc == 2), perf_mode=DR,
                            )
                        out_sl = dst[:, half * 512 : (half + 1) * 512]
                        if name == "r":
                            # keep 32x scale in r (folded into og8)
                            if zero_bias:
                                nc.vector.tensor_copy(out_sl, ps[:])
                            else:
                                nc.vector.tensor_scalar(
                                    out=out_sl, in0=ps[:],
                                    scalar1=bqkr_t[:, si, h : h + 1], scalar2=None,
                                    op0=ALU.add,
                                )
                        else:
                            if zero_bias:
                                nc.vector.tensor_scalar(
                                    out=out_sl, in0=ps[:],
                                    scalar1=INV_WS, scalar2=None, op0=ALU.mult,
                                )
                            else:
                                nc.vector.tensor_scalar(
                                    out=out_sl, in0=ps[:],
                                    scalar1=INV_WS, scalar2=bqkr_t[:, si, h : h + 1],
                                    op0=ALU.mult, op1=ALU.add,
                                )
                return qkrT

            def energy_stage(h, qkrT):
                expE = exp_pool.tile([128, NQT, N], FP8, tag="expE", name="expE")
                for kt in range(NQT):
                    ep = eng_ps.tile([128, N], F32, tag="ep", name="ep")
                    for qh in range(2):
                        nc.tensor.matmul(
                            ep[:, qh * 512 : (qh + 1) * 512],
                            qkrT["k"][:, kt * 128 : (kt + 1) * 128],
                            qkrT["q"][:, qh * 512 : (qh + 1) * 512],
                            start=True, stop=True, skip_group_check=True,
                        )
                    nc.scalar.activation(
                        out=expE[:, kt, :], in_=ep[:], func=AF.Exp,
                        bias=shift_t[:], scale=1.0,
                    )
                return expE

            def attv_stage(h, expE):
                op_ = att_ps.tile([128, N], F32, tag="op", name="op")
                for j in range(NQT // 2):
                    for qh in range(2):
                        nc.tensor.matmul(
                            op_[:, qh * 512 : (qh + 1) * 512],
                            v_aug[:, h, 2 * j : 2 * j + 2, :],
                            expE[:, 2 * j : 2 * j + 2, qh * 512 : (qh + 1) * 512],
                            start=(j == 0), stop=(j == NQT // 2 - 1),
                            perf_mode=DR, skip_group_check=True,
                        )
                return op_

            def tail_stage(h, op_, qkrT):
                # og8[:, h, :] = (op_[0:D] * r_sb) / (32 * op_[D]) -- with eps
                su = tmp_pool.tile([1, N], F32, tag="su", name="su")
                nc.vector.tensor_scalar(
                    out=su[:], in0=op_[D : D + 1, :],
                    scalar1=32.0, scalar2=SUM_EPS, op0=ALU.mult, op1=ALU.add,
                )
                gated = tmp_pool.tile([D, N], F32, tag="gated", name="gated")
                nc.vector.tensor_tensor(
                    out=gated[:], in0=op_[0:D, :], in1=qkrT["r"][:], op=ALU.mult,
                )
                rcp = tmp_pool.tile([1, N], F32, tag="rcp", name="rcp")
                nc.vector.reciprocal_approx_fast(rcp[:], su[:])
                bcr = tmp_pool.tile([D, N], F32, tag="bcr", name="bcr")
                nc.gpsimd.partition_broadcast(bcr[:], rcp[:])
                nc.vector.tensor_tensor(
                    out=og8[:, h, :], in0=gated[:], in1=bcr[:], op=ALU.mult,
                )

            prev = None  # (h, expE, qkrT)
            for h in range(H):
                qkrT = qkr_stage(h)
                if prev is not None:
                    ph, pexpE, pqkrT = prev
                    op_ = attv_stage(ph, pexpE)
                    tail_stage(ph, op_, pqkrT)
                expE = energy_stage(h, qkrT)
                prev = (h, expE, qkrT)
            ph, pexpE, pqkrT = prev
            op_ = attv_stage(ph, pexpE)
            tail_stage(ph, op_, pqkrT)

        # ---- phase D: proj (fp8 DR) + LN1 -> x1b (bf16) + x1T8 (fp8) ----
        x1b = x1_pool.tile([128, NQT, E], BF16, name="x1b")
        x1T8 = x1t_pool.tile([128, NEC, N], FP8, name="x1T8")
        with (
            tc.tile_pool(name="bcmid", bufs=1) as bcm_pool,
            tc.tile_pool(name="ln_tmp", bufs=2) as ln_pool,
            tc.tile_pool(name="y1_ps", bufs=2, space="PSUM") as y1_ps,
            tc.tile_pool(name="tp1_ps", bufs=2, space="PSUM") as tp1_ps,
        ):
            bcm = None
            if not (identity_ln and zero_bias):
                bcm = bcm_pool.tile([128, 3, E], F32, name="bcm")
                for i, d in enumerate((bproj_d, ln1g_d, ln1b_d)):
                    _bcast_dma(nc, bcm[:, i, :], d[0:1, :])
            for qt in range(NQT):
                yp = y1_ps.tile([128, 1024], F32, tag="yp", name="yp")
                for o, w in ((0, 512), (512, 256)):
                    for hp in range(4):
                        nc.tensor.matmul(
                            yp[:, o : o + w],
                            og8[:, 2 * hp : 2 * hp + 2, qt * 128 : (qt + 1) * 128],
                            wp8[:, hp, :, o : o + w],
                            start=(hp == 0), stop=(hp == 3), perf_mode=DR,
                        )
                t1 = ln_pool.tile([128, E], F32, tag="t1", name="t1")
                xr = xb16[:, qt, :]
                if not zero_bias:
                    xrf = ln_pool.tile([128, E], F32, tag="xrf", name="xrf")
                    nc.gpsimd.tensor_tensor(out=xrf[:], in0=xr, in1=bcm[:, 0, :], op=ALU.add)
                    xr = xrf[:]
                nc.vector.scalar_tensor_tensor(
                    out=t1[:], in0=yp[:, 0:E], scalar=PROJ_DEQ, in1=xr,
                    op0=ALU.mult, op1=ALU.add,
                )
                xo = x1b[:, qt, :]
                _ln_stats_norm(nc, ln_pool, t1[:], xo, eps_t,
                               bcm[:, 1, :] if bcm is not None else None,
                               bcm[:, 2, :] if bcm is not None else None,
                               identity_ln, "ln1")
                pt1 = tp1_ps.tile([128, NEC, 128], BF16, tag="pt1", name="pt1")
                for ec in range(NEC):
                    nc.tensor.transpose(pt1[:, ec, :], x1b[:, qt, ec * 128 : (ec + 1) * 128], identb[:])
                if qt % 2 == 0:
                    nc.vector.tensor_copy(x1T8[:, :, qt * 128 : (qt + 1) * 128], pt1[:])
                else:
                    nc.scalar.activation(
                        out=x1T8[:, :, qt * 128 : (qt + 1) * 128], in_=pt1[:], func=AF.Copy,
                    )

        # ---- phase E: ffn + residual + LN2 -> y ----
        with (
            tc.tile_pool(name="bcend", bufs=1) as bce_pool,
            tc.tile_pool(name="w2sb", bufs=1) as w2_pool,
            tc.tile_pool(name="gstore", bufs=2) as gs_pool,
            tc.tile_pool(name="ln2_tmp", bufs=2) as ln2_pool,
            tc.tile_pool(name="out", bufs=2) as out_pool,
        ):
            bce = None
            if not (identity_ln and zero_bias):
                bce = bce_pool.tile([128, 3, E], F32, name="bce")
                for i, d in enumerate((bff2_d, ln2g_d, ln2b_d)):
                    _bcast_dma(nc, bce[:, i, :], d[0:1, :])
            w2sb = w2_pool.tile([128, NCT, E], BF16, name="w2sb")
            for ct in range(NCT):
                nc.sync.dma_start(out=w2sb[:, ct, :], in_=wff2_d[ct * 128 : (ct + 1) * 128, :])

            gTs = []
            with tc.tile_pool(name="h1_ps", bufs=3, space="PSUM") as h1_ps:
                for half in range(2):
                    gT = gs_pool.tile([128, NCT, 512], BF16, tag="gT", name="gT")
                    gTs.append(gT)
                    for ct in range(NCT):
                        hp_ = h1_ps.tile([128, 512], F32, tag="h1", name="h1")
                        for kc in range(3):
                            nc.tensor.matmul(
                                hp_[:],
                                wff1sb[:, ct, kc],
                                x1T8[:, 2 * kc : 2 * kc + 2,
                                     half * 512 : (half + 1) * 512],
                                start=(kc == 0), stop=(kc == 2), perf_mode=DR,
                            )
                        nc.scalar.activation(
                            out=gT[:, ct, :], in_=hp_[:], func=AF.Gelu,
                            bias=bff1_t[:, ct : ct + 1], scale=INV_WS,
                        )
                with tc.tile_pool(name="y2_ps", bufs=2, space="PSUM") as y2_ps:
                    for half in range(2):
                        gT = gTs[half]
                        for iq in range(4):
                            qt = half * 4 + iq
                            y2p = y2_ps.tile([128, 1024], F32, tag="y2", name="y2")
                            for o, w in ((0, 512), (512, 256)):
                                for ct in range(NCT):
                                    nc.tensor.matmul(
                                        y2p[:, o : o + w],
                                        gT[:, ct, iq * 128 : (iq + 1) * 128],
                                        w2sb[:, ct, o : o + w],
                                        start=(ct == 0), stop=(ct == NCT - 1),
                                    )
                            t2 = ln2_pool.tile([128, E], F32, tag="t2", name="t2")
                            x1q = x1b[:, qt, :]
                            if not zero_bias:
                                x1f = ln2_pool.tile([128, E], F32, tag="x1f", name="x1f")
                                nc.gpsimd.tensor_tensor(out=x1f[:], in0=x1q, in1=bce[:, 0, :], op=ALU.add)
                                nc.vector.tensor_tensor(out=t2[:], in0=y2p[:, 0:E], in1=x1f[:], op=ALU.add)
                            else:
                                nc.vector.tensor_tensor(out=t2[:], in0=y2p[:, 0:E], in1=x1q, op=ALU.add)
                            yout = out_pool.tile([128, E], F32, tag="yout", name="yout")
                            _ln_stats_norm(nc, ln2_pool, t2[:], yout[:], eps_t,
                                           bce[:, 1, :] if bce is not None else None,
                                           bce[:, 2, :] if bce is not None else None,
                                           identity_ln, "ln2")
                            nc.sync.dma_start(out=y_d[qt * 128 : (qt + 1) * 128, :], in_=yout[:])

    nc.compile()
    return nc


_NC_CACHE = {}


def _get_nc(identity_ln=False, zero_bias=False):
    key = (identity_ln, zero_bias)
    if key not in _NC_CACHE:
        _NC_CACHE[key] = _build(identity_ln, zero_bias)
    return _NC_CACHE[key]


def _q8(a):
    return np.asarray(a, dtype=ml_dtypes.float8_e4m3fn)


def _prep_weights(w_qkvr, b_qkvr, w_proj, b_proj, ln1_g, ln1_b,
                  w_ff1, b_ff1, w_ff2, b_ff2, ln2_g, ln2_b):
    w4 = np.asarray(w_qkvr, np.float32).reshape(E, H, D, 4)
    b4 = np.asarray(b_qkvr, np.float32).reshape(H, D, 4)

    # wqkr: [p, h, t, kc, i, d] = 32*w_t[256kc+128i+p, h, :]
    wqkr = np.zeros((128, H, 3, 3, 2, D), np.float32)
    for ti, t in enumerate((0, 1, 3)):
        wt = w4[..., t] * WS  # [E, H, D]
        for kc in range(3):
            for i in range(2):
                wqkr[:, :, ti, kc, i, :] = wt[256 * kc + 128 * i : 256 * kc + 128 * (i + 1)]
    # wv: [p, kc, i, n] = 32*wv[256kc+128i+p, n]
    wvf = w4[..., 2].reshape(E, E) * WS
    wv = np.zeros((128, 3, 2, E), np.float32)
    for kc in range(3):
        for i in range(2):
            wv[:, kc, i, :] = wvf[256 * kc + 128 * i : 256 * kc + 128 * (i + 1)]
    # wproj: [p(d), hp, i, n] = 32*wproj[(2hp+i)*96+p, n]
    wpf = np.asarray(w_proj, np.float32) * WS
    wp = np.zeros((D, 4, 2, E), np.float32)
    for hp in range(4):
        for i in range(2):
            wp[:, hp, i, :] = wpf[(2 * hp + i) * D : (2 * hp + i + 1) * D]
    # biases: q,k true; r pre-scaled by 32 (r keeps 32x);
    # v bias pre-scaled by VA_SCALE/sqrt(E) (folded into v_aug evac)
    bqkr = np.stack([b4[..., 0], b4[..., 1], b4[..., 3] * WS], 0).transpose(2, 0, 1)
    bv = np.ascontiguousarray(
        (b4[..., 2] * (VA_SCALE / np.sqrt(E))).reshape(1, E)).astype(np.float32)
    # wff1: [p, ct, kc, i, c] = 32*w_ff1[256kc+128i+p, 128ct+c]
    w1f = np.asarray(w_ff1, np.float32) * WS
    wff1 = np.zeros((128, NCT, 3, 2, 128), np.float32)
    w1r = w1f.reshape(3, 2, 128, NCT, 128)  # [kc, i, p, ct, c]
    wff1[:] = w1r.transpose(2, 3, 0, 1, 4)
    bff1 = np.ascontiguousarray(np.asarray(b_ff1, np.float32).reshape(NCT, 128).T)
    return {
        "wqkr": _q8(wqkr), "wv": _q8(wv), "wproj": _q8(wp),
        "bqkr": np.ascontiguousarray(bqkr), "bv": bv,
        "bproj": np.asarray(b_proj, np.float32).reshape(1, E).copy(),
        "ln1g": np.asarray(ln1_g, np.float32).reshape(1, E).copy(),
        "ln1b": np.asarray(ln1_b, np.float32).reshape(1, E).copy(),
        "wff1": _q8(wff1), "bff1": bff1,
        "wff2": np.ascontiguousarray(np.asarray(w_ff2, np.float32)).astype(ml_dtypes.bfloat16),
        "bff2": np.asarray(b_ff2, np.float32).reshape(1, E).copy(),
        "ln2g": np.asarray(ln2_g, np.float32).reshape(1, E).copy(),
        "ln2b": np.asarray(ln2_b, np.float32).reshape(1, E).copy(),
    }


def _in_maps(inputs):
    x = np.asarray(inputs["x"], np.float32)
    shared = _prep_weights(
        inputs["w_qkvr"], inputs["b_qkvr"], inputs["w_proj"], inputs["b_proj"],
        inputs["ln1_g"], inputs["ln1_b"], inputs["w_ff1"], inputs["b_ff1"],
        inputs["w_ff2"], inputs["b_ff2"], inputs["ln2_g"], inputs["ln2_b"],
    )
    maps = []
    for i in range(N_CORES):
        xi = x[i]
        x8 = np.asarray(xi, ml_dtypes.float8_e4m3fn)
        # x8t[p, ec, n] = x8[n, 128*ec + p]
        x8t = np.ascontiguousarray(x8.T.reshape(NEC, 128, N).transpose(1, 0, 2))
        xb16 = np.ascontiguousarray(xi.astype(ml_dtypes.bfloat16))
        maps.append({**shared, "x8t": x8t, "xb16": xb16})
    return maps


def _flags(inputs):
    z = lambda k: not np.any(np.asarray(inputs[k]))
    one = lambda k: bool(np.all(np.asarray(inputs[k]) == 1.0))
    identity_ln = (one("ln1_g") and z("ln1_b") and one("ln2_g") and z("ln2_b"))
    zero_bias = (z("b_qkvr") and z("b_proj") and z("b_ff2"))
    return identity_ln, zero_bias


def kernel(**inputs) -> np.ndarray:
    identity_ln, zero_bias = _flags(inputs)
    nc = _get_nc(identity_ln, zero_bias)
    res = run_bass_kernel_spmd(nc, _in_maps(inputs), core_ids=list(range(N_CORES)))
    return np.stack([res.results[i]["y"] for i in range(N_CORES)], axis=0)
